# revision 11
# baseline (speedup 1.0000x reference)
"""FlowNetC-style correlation (cost volume) kernel for Trainium2.

Input : feat1, feat2  [B=8, H=128, W=256, C=128] fp32
Output: [B, H, W, 81]  -- out[b,h,w,dy*9+dx] = sum_c f1[b,h,w,c] * f2p[b,h+dy,w+dx,c]
        where f2p is feat2 zero-padded by 4 on each spatial side.

Per NeuronCore (batch-sharded, 1 image/core):
  - Host pre-casts inputs to fp16 (halves HBM traffic; tolerance is 2e-2).
  - Staging DMA loads pixel-PAIRS per partition (512B contiguous runs);
    PE transposes build f1T [c, hl*256+w] and zero-padded f2pT [c, 136, 264]
    (fp16 PSUM, evicted with a stride-2 even/odd de-interleave).
  - Correlation per (h, w-half): 4 column-group matmuls (tile_position
    (0,32A), M=32 pixels, K=C=128, N=9*40=360, dy-major so the rhs streams
    40 contiguous elems per dy row).  psum[32A+m, dy*40+j] =
    corr(pixel w=wh*128+32A+m, dy, dx=j-m).  Two h rows share one 2-bank
    psum tile and are evicted to fp16 with a single copy.
  - The RAW BAND is DMAd out in big contiguous transfers; the host extracts
    the 9x9 window per pixel (j = f..f+8 diagonal) with one as_strided view
    + one cast/reorder pass.
"""

import sys

if '/opt/trn_rl_repo' not in sys.path:
    sys.path.insert(0, '/opt/trn_rl_repo')

import numpy as np

import concourse.bacc as bacc
import concourse.mybir as mybir
from concourse import masks
from concourse.bass_utils import run_bass_kernel_spmd
from concourse.tile import TileContext

H, W, C = 128, 256, 128
D = 9                      # displacement window 9x9
HP, WP = H + 8, W + 8      # padded f2 spatial dims (136, 264)
JW = 40                    # band width per 32-pixel strip (32 + 9 - 1)
NW = JW * D                # 360 = matmul N per (h, w-half)
HB = 8                     # h rows per band block
ROW_E = HB * NW            # 2880 elements per partition per E block
F32 = mybir.dt.float32
F16 = mybir.dt.float16

_CACHED_NC = None


def _build():
    nc = bacc.Bacc("TRN2", target_bir_lowering=False, debug=False,
                   num_devices=1)
    f1_d = nc.dram_tensor("feat1", [H, W, C], F16, kind="ExternalInput")
    f2_d = nc.dram_tensor("feat2", [H, W, C], F16, kind="ExternalInput")
    # raw correlation band, extracted host-side
    band_d = nc.dram_tensor("band", [2, H // HB, 128, ROW_E], F16,
                            kind="ExternalOutput")

    with TileContext(nc) as tc:
        with (
            tc.tile_pool(name="const", bufs=1) as constp,
            tc.tile_pool(name="big", bufs=1) as bigp,
            tc.tile_pool(name="stag", bufs=6) as stagp,
            tc.tile_pool(name="f1t", bufs=16) as f1tp,
            tc.tile_pool(name="ebuf", bufs=4) as ep,
            tc.tile_pool(name="pst", bufs=2, space="PSUM") as pstp,
            tc.tile_pool(name="psc", bufs=2, space="PSUM") as pscp,
        ):
            ident = constp.tile([128, 128], F16)
            masks.make_identity(nc, ident[:, :])

            f2pT = bigp.tile([128, HP, WP], F16)     # 70.1KB/partition
            # zero only the 4-wide pad borders
            nc.vector.memset(f2pT[:, 0:4, :], 0.0)
            nc.vector.memset(f2pT[:, HP - 4:HP, :], 0.0)
            nc.vector.memset(f2pT[:, 4:HP - 4, 0:4], 0.0)
            nc.vector.memset(f2pT[:, 4:HP - 4, WP - 4:WP], 0.0)

            f1t_blocks = [None] * 16

            def stage_a(blk):
                """Load 8 image rows of f1/f2, PE-transpose to fp16."""
                h0 = blk * 8
                stag1 = stagp.tile([128, 8, 256], F16, tag="stag")
                stag2 = stagp.tile([128, 8, 256], F16, tag="stag")
                # pixel-pair layout: partition p <- pixels (2p, 2p+1);
                # per-partition runs are 256 contiguous elems (512B).
                src1 = f1_d[h0:h0 + 8, :, :].rearrange(
                    "h (p e) c -> p h (e c)", p=128)
                nc.sync.dma_start(out=stag1[:, :, :], in_=src1)
                src2 = f2_d[h0:h0 + 8, :, :].rearrange(
                    "h (p e) c -> p h (e c)", p=128)
                nc.sync.dma_start(out=stag2[:, :, :], in_=src2)

                f1tb = f1tp.tile([128, 8 * 256], F16, tag="f1t")
                f1t_blocks[blk] = f1tb
                for half in range(2):
                    pst = pstp.tile([128, 8, 128], F16, tag="pst")
                    for q in range(8):       # q = (hl, e): tile of row h0+hl
                        hl = half * 4 + q // 2
                        e = q % 2
                        nc.tensor.transpose(
                            pst[:, q, :],
                            stag1[:, hl, e * 128:(e + 1) * 128],
                            ident[:, :])
                    # de-interleave even/odd pixels: elem (hl,e,k)->hl*256+2k+e
                    dst1 = f1tb[:, half * 1024:(half + 1) * 1024].rearrange(
                        "c (a k e) -> c a e k", a=4, k=128, e=2)
                    src1 = pst[:, :, :].rearrange("c (a e) k -> c a e k", e=2)
                    if (blk + half) % 2 == 0:
                        nc.scalar.copy(dst1, src1)
                    else:
                        nc.vector.tensor_copy(dst1, src1)

                for half in range(2):
                    pst = pstp.tile([128, 8, 128], F16, tag="pst")
                    for q in range(8):
                        hl = half * 4 + q // 2
                        e = q % 2
                        nc.tensor.transpose(
                            pst[:, q, :],
                            stag2[:, hl, e * 128:(e + 1) * 128],
                            ident[:, :])
                    r0 = h0 + 4 + half * 4
                    dst2 = f2pT[:, r0:r0 + 4, 4:260].rearrange(
                        "c a (k e) -> c a e k", k=128, e=2)
                    src2 = pst[:, :, :].rearrange("c (a e) b -> c a e b", e=2)
                    if (blk + half) % 2 == 0:
                        nc.vector.tensor_copy(dst2, src2)
                    else:
                        nc.scalar.copy(dst2, src2)

            def stage_b(hblk):
                """Correlate 8 h rows, dump the raw band per w-half."""
                for wh in range(2):
                    E = ep.tile([128, ROW_E], F16, tag="ebuf")
                    hl0 = 0
                    for hp, nr in enumerate((3, 3, 2)):
                        ps = pscp.tile([128, 3, 512], F32, tag="psc")
                        for r in range(nr):
                            hl = hl0 + r
                            h = hblk * HB + hl
                            f1tb = f1t_blocks[h // 8]
                            base = (h % 8) * 256 + wh * 128
                            for A in range(4):
                                lhsT = f1tb[:, base + 32 * A:
                                            base + 32 * A + 32]
                                w0 = wh * 128 + 32 * A
                                rhs = f2pT[:, h:h + D, w0:w0 + JW]
                                nc.tensor.matmul(
                                    ps[32 * A:32 * A + 32, r, 0:NW],
                                    lhsT, rhs, start=True, stop=True,
                                    tile_position=(0, 32 * A))
                        dst = E[:, hl0 * NW:(hl0 + nr) * NW].rearrange(
                            "c (r n) -> c r n", r=nr)
                        if (wh * 3 + hp) % 2 == 0:
                            nc.scalar.copy(dst, ps[:, 0:nr, 0:NW])
                        else:
                            nc.vector.tensor_copy(dst, ps[:, 0:nr, 0:NW])
                        hl0 += nr
                    nc.sync.dma_start(out=band_d[wh, hblk, :, :],
                                      in_=E[:, :])

            # software-pipelined emission: B(k) needs A(k) and A(k+1)
            stage_a(0)
            stage_a(1)
            for hblk in range(16):
                if hblk + 2 < 16:
                    stage_a(hblk + 2)
                stage_b(hblk)

    nc.compile()
    return nc


def kernel(feat1: np.ndarray, feat2: np.ndarray) -> np.ndarray:
    global _CACHED_NC
    feat1 = np.asarray(feat1, dtype=np.float16)
    feat2 = np.asarray(feat2, dtype=np.float16)
    B = feat1.shape[0]
    if _CACHED_NC is None:
        _CACHED_NC = _build()
    nc = _CACHED_NC
    in_maps = [{"feat1": np.ascontiguousarray(feat1[b]),
                "feat2": np.ascontiguousarray(feat2[b])} for b in range(B)]
    res = run_bass_kernel_spmd(nc, in_maps, core_ids=list(range(B)))
    band = np.stack([res.results[b]["band"] for b in range(B)], axis=0)
    # band[b, wh, hblk, p=(g,f), (hl, dy, j)]; pixel w = wh*128+32g+f uses
    # columns j = f..f+8 of its group's band at each dy.
    A = band.reshape(B, 2, H // HB, 4, 32, HB, D, JW)
    s = A.strides
    Cv = np.lib.stride_tricks.as_strided(
        A, shape=(B, 2, H // HB, 4, 32, HB, D, D),
        strides=(s[0], s[1], s[2], s[3], s[4] + s[7], s[5], s[6], s[7]))
    # Cv[b, wh, hblk, g, f, hl, dy, dx] -> out[b, h, w, dy*9+dx]
    out = Cv.transpose(0, 2, 5, 1, 3, 4, 6, 7)
    return np.ascontiguousarray(out, dtype=np.float32).reshape(B, H, W, 81)


# revision 13
# speedup vs baseline: 1.1133x; 1.1133x over previous
"""FlowNetC-style correlation (cost volume) kernel for Trainium2.

Input : feat1, feat2  [B=8, H=128, W=256, C=128] fp32
Output: [B, H, W, 81]  -- out[b,h,w,dy*9+dx] = sum_c f1[b,h,w,c] * f2p[b,h+dy,w+dx,c]
        where f2p is feat2 zero-padded by 4 on each spatial side.

Per NeuronCore (batch-sharded, 1 image/core):
  - Host pre-casts inputs to fp16 (halves HBM traffic; tolerance is 2e-2).
  - Staging DMA loads pixel-PAIRS per partition (512B contiguous runs);
    PE transposes build f1T [c, hl*256+w] and zero-padded f2pT [c, 136, 264]
    (fp16 PSUM, evicted with a stride-2 even/odd de-interleave).
  - Correlation per (h, w-half): 4 column-group matmuls (tile_position
    (0,32A), M=32 pixels, K=C=128, N=9*40=360, dy-major so the rhs streams
    40 contiguous elems per dy row).  psum[32A+m, dy*40+j] =
    corr(pixel w=wh*128+32A+m, dy, dx=j-m).  Two h rows share one 2-bank
    psum tile and are evicted to fp16 with a single copy.
  - The RAW BAND is DMAd out in big contiguous transfers; the host extracts
    the 9x9 window per pixel (j = f..f+8 diagonal) with one as_strided view
    + one cast/reorder pass.
"""

import sys

if '/opt/trn_rl_repo' not in sys.path:
    sys.path.insert(0, '/opt/trn_rl_repo')

import numpy as np

import concourse.bacc as bacc
import concourse.mybir as mybir
from concourse import masks
from concourse.bass_utils import run_bass_kernel_spmd
from concourse.tile import TileContext

H, W, C = 128, 256, 128
D = 9                      # displacement window 9x9
HP, WP = H + 8, W + 8      # padded f2 spatial dims (136, 264)
JW = 40                    # band width per 32-pixel strip (32 + 9 - 1)
NW = JW * D                # 360 = matmul N per (h, w-half)
HB = 8                     # h rows per band block
ROW_E = HB * NW            # 2880 elements per partition per E block
F32 = mybir.dt.float32
F16 = mybir.dt.float16

_CACHED_NC = None


def _build():
    nc = bacc.Bacc("TRN2", target_bir_lowering=False, debug=False,
                   num_devices=1)
    f1_d = nc.dram_tensor("feat1", [H, W, C], F16, kind="ExternalInput")
    f2_d = nc.dram_tensor("feat2", [H, W, C], F16, kind="ExternalInput")
    # raw correlation band, extracted host-side
    band_d = nc.dram_tensor("band", [2, H // HB, 128, ROW_E], F16,
                            kind="ExternalOutput")

    with TileContext(nc) as tc:
        with (
            tc.tile_pool(name="const", bufs=1) as constp,
            tc.tile_pool(name="big", bufs=1) as bigp,
            tc.tile_pool(name="stag", bufs=6) as stagp,
            tc.tile_pool(name="f1t", bufs=16) as f1tp,
            tc.tile_pool(name="ebuf", bufs=4) as ep,
            tc.tile_pool(name="pst", bufs=2, space="PSUM") as pstp,
            tc.tile_pool(name="psc", bufs=3, space="PSUM") as pscp,
        ):
            ident = constp.tile([128, 128], F16)
            masks.make_identity(nc, ident[:, :])

            f2pT = bigp.tile([128, HP, WP], F16)     # 70.1KB/partition
            # zero only the 4-wide pad borders
            nc.vector.memset(f2pT[:, 0:4, :], 0.0)
            nc.vector.memset(f2pT[:, HP - 4:HP, :], 0.0)
            nc.vector.memset(f2pT[:, 4:HP - 4, 0:4], 0.0)
            nc.vector.memset(f2pT[:, 4:HP - 4, WP - 4:WP], 0.0)

            f1t_blocks = [None] * 16

            def stage_a(blk):
                """Load 8 image rows of f1/f2, PE-transpose to fp16."""
                h0 = blk * 8
                stag1 = stagp.tile([128, 8, 256], F16, tag="stag")
                stag2 = stagp.tile([128, 8, 256], F16, tag="stag")
                # pixel-pair layout: partition p <- pixels (2p, 2p+1);
                # per-partition runs are 256 contiguous elems (512B).
                src1 = f1_d[h0:h0 + 8, :, :].rearrange(
                    "h (p e) c -> p h (e c)", p=128)
                nc.sync.dma_start(out=stag1[:, :, :], in_=src1)
                src2 = f2_d[h0:h0 + 8, :, :].rearrange(
                    "h (p e) c -> p h (e c)", p=128)
                nc.sync.dma_start(out=stag2[:, :, :], in_=src2)

                f1tb = f1tp.tile([128, 8 * 256], F16, tag="f1t")
                f1t_blocks[blk] = f1tb
                for half in range(2):
                    pst = pstp.tile([128, 8, 128], F16, tag="pst")
                    for q in range(8):       # q = (hl, e): tile of row h0+hl
                        hl = half * 4 + q // 2
                        e = q % 2
                        nc.tensor.transpose(
                            pst[:, q, :],
                            stag1[:, hl, e * 128:(e + 1) * 128],
                            ident[:, :])
                    # de-interleave even/odd pixels: elem (hl,e,k)->hl*256+2k+e
                    dst1 = f1tb[:, half * 1024:(half + 1) * 1024].rearrange(
                        "c (a k e) -> c a e k", a=4, k=128, e=2)
                    src1 = pst[:, :, :].rearrange("c (a e) k -> c a e k", e=2)
                    if (blk + half) % 2 == 0:
                        nc.scalar.copy(dst1, src1)
                    else:
                        nc.vector.tensor_copy(dst1, src1)

                for half in range(2):
                    pst = pstp.tile([128, 8, 128], F16, tag="pst")
                    for q in range(8):
                        hl = half * 4 + q // 2
                        e = q % 2
                        nc.tensor.transpose(
                            pst[:, q, :],
                            stag2[:, hl, e * 128:(e + 1) * 128],
                            ident[:, :])
                    r0 = h0 + 4 + half * 4
                    dst2 = f2pT[:, r0:r0 + 4, 4:260].rearrange(
                        "c a (k e) -> c a e k", k=128, e=2)
                    src2 = pst[:, :, :].rearrange("c (a e) b -> c a e b", e=2)
                    if (blk + half) % 2 == 0:
                        nc.vector.tensor_copy(dst2, src2)
                    else:
                        nc.scalar.copy(dst2, src2)

            def stage_b(hblk):
                """Correlate 8 h rows, dump the raw band per w-half."""
                for wh in range(2):
                    E = ep.tile([128, ROW_E], F16, tag="ebuf")
                    for hp in range(HB // 2):
                        ps = pscp.tile([128, 2, 512], F32, tag="psc")
                        for r in range(2):
                            hl = hp * 2 + r
                            h = hblk * HB + hl
                            f1tb = f1t_blocks[h // 8]
                            base = (h % 8) * 256 + wh * 128
                            for A in range(4):
                                lhsT = f1tb[:, base + 32 * A:
                                            base + 32 * A + 32]
                                w0 = wh * 128 + 32 * A
                                rhs = f2pT[:, h:h + D, w0:w0 + JW]
                                nc.tensor.matmul(
                                    ps[32 * A:32 * A + 32, r, 0:NW],
                                    lhsT, rhs, start=True, stop=True,
                                    tile_position=(0, 32 * A))
                        dst = E[:, hp * 2 * NW:(hp + 1) * 2 * NW].rearrange(
                            "c (r n) -> c r n", r=2)
                        if hp % 2 == 0:
                            nc.scalar.copy(dst, ps[:, :, 0:NW])
                        else:
                            nc.vector.tensor_copy(dst, ps[:, :, 0:NW])
                    nc.sync.dma_start(out=band_d[wh, hblk, :, :],
                                      in_=E[:, :])

            # software-pipelined emission: B(k) needs A(k) and A(k+1)
            stage_a(0)
            stage_a(1)
            for hblk in range(16):
                if hblk + 2 < 16:
                    stage_a(hblk + 2)
                stage_b(hblk)

    nc.compile()
    return nc


def kernel(feat1: np.ndarray, feat2: np.ndarray) -> np.ndarray:
    global _CACHED_NC
    feat1 = np.asarray(feat1, dtype=np.float16)
    feat2 = np.asarray(feat2, dtype=np.float16)
    B = feat1.shape[0]
    if _CACHED_NC is None:
        _CACHED_NC = _build()
    nc = _CACHED_NC
    in_maps = [{"feat1": np.ascontiguousarray(feat1[b]),
                "feat2": np.ascontiguousarray(feat2[b])} for b in range(B)]
    res = run_bass_kernel_spmd(nc, in_maps, core_ids=list(range(B)))
    band = np.stack([res.results[b]["band"] for b in range(B)], axis=0)
    # band[b, wh, hblk, p=(g,f), (hl, dy, j)]; pixel w = wh*128+32g+f uses
    # columns j = f..f+8 of its group's band at each dy.
    A = band.reshape(B, 2, H // HB, 4, 32, HB, D, JW)
    s = A.strides
    Cv = np.lib.stride_tricks.as_strided(
        A, shape=(B, 2, H // HB, 4, 32, HB, D, D),
        strides=(s[0], s[1], s[2], s[3], s[4] + s[7], s[5], s[6], s[7]))
    # Cv[b, wh, hblk, g, f, hl, dy, dx] -> out[b, h, w, dy*9+dx]
    out = Cv.transpose(0, 2, 5, 1, 3, 4, 6, 7)
    return np.ascontiguousarray(out, dtype=np.float32).reshape(B, H, W, 81)


# revision 14
# speedup vs baseline: 1.1214x; 1.0072x over previous
"""FlowNetC-style correlation (cost volume) kernel for Trainium2.

Input : feat1, feat2  [B=8, H=128, W=256, C=128] fp32
Output: [B, H, W, 81]  -- out[b,h,w,dy*9+dx] = sum_c f1[b,h,w,c] * f2p[b,h+dy,w+dx,c]
        where f2p is feat2 zero-padded by 4 on each spatial side.

Per NeuronCore (batch-sharded, 1 image/core):
  - Host pre-casts inputs to fp16 (halves HBM traffic; tolerance is 2e-2).
  - Staging DMA loads pixel-PAIRS per partition (512B contiguous runs);
    PE transposes build f1T [c, hl*256+w] and zero-padded f2pT [c, 136, 264]
    (fp16 PSUM, evicted with a stride-2 even/odd de-interleave).
  - Correlation per (h, w-half): 4 column-group matmuls (tile_position
    (0,32A), M=32 pixels, K=C=128, N=9*40=360, dy-major so the rhs streams
    40 contiguous elems per dy row).  psum[32A+m, dy*40+j] =
    corr(pixel w=wh*128+32A+m, dy, dx=j-m).  Two h rows share one 2-bank
    psum tile and are evicted to fp16 with a single copy.
  - The RAW BAND is DMAd out in big contiguous transfers; the host extracts
    the 9x9 window per pixel (j = f..f+8 diagonal) with one as_strided view
    + one cast/reorder pass.
"""

import sys

if '/opt/trn_rl_repo' not in sys.path:
    sys.path.insert(0, '/opt/trn_rl_repo')

import numpy as np

import concourse.bacc as bacc
import concourse.mybir as mybir
from concourse import masks
from concourse.bass_utils import run_bass_kernel_spmd
from concourse.tile import TileContext

H, W, C = 128, 256, 128
D = 9                      # displacement window 9x9
HP, WP = H + 8, W + 8      # padded f2 spatial dims (136, 264)
JW = 40                    # band width per 32-pixel strip (32 + 9 - 1)
NW = JW * D                # 360 = matmul N per (h, w-half)
HB = 8                     # h rows per band block
ROW_E = HB * NW            # 2880 elements per partition per E block
F32 = mybir.dt.float32
F16 = mybir.dt.float16

_CACHED_NC = None


def _build():
    nc = bacc.Bacc("TRN2", target_bir_lowering=False, debug=False,
                   num_devices=1)
    f1_d = nc.dram_tensor("feat1", [H, W, C], F16, kind="ExternalInput")
    f2_d = nc.dram_tensor("feat2", [H, W, C], F16, kind="ExternalInput")
    # raw correlation band, extracted host-side
    band_d = nc.dram_tensor("band", [2, H // HB, 128, ROW_E], F16,
                            kind="ExternalOutput")

    with TileContext(nc) as tc:
        with (
            tc.tile_pool(name="const", bufs=1) as constp,
            tc.tile_pool(name="big", bufs=1) as bigp,
            tc.tile_pool(name="stag", bufs=6) as stagp,
            tc.tile_pool(name="f1t", bufs=16) as f1tp,
            tc.tile_pool(name="ebuf", bufs=4) as ep,
            tc.tile_pool(name="pst", bufs=2, space="PSUM") as pstp,
            tc.tile_pool(name="psc", bufs=3, space="PSUM") as pscp,
        ):
            ident = constp.tile([128, 128], F16)
            masks.make_identity(nc, ident[:, :])

            f2pT = bigp.tile([128, HP, WP], F16)     # 70.1KB/partition
            # zero only the 4-wide pad borders
            nc.vector.memset(f2pT[:, 0:4, :], 0.0)
            nc.vector.memset(f2pT[:, HP - 4:HP, :], 0.0)
            nc.vector.memset(f2pT[:, 4:HP - 4, 0:4], 0.0)
            nc.vector.memset(f2pT[:, 4:HP - 4, WP - 4:WP], 0.0)

            f1t_blocks = [None] * 16

            def stage_a(blk):
                """Load 8 image rows of f1/f2, PE-transpose to fp16."""
                h0 = blk * 8
                stag1 = stagp.tile([128, 8, 256], F16, tag="stag")
                stag2 = stagp.tile([128, 8, 256], F16, tag="stag")
                # pixel-pair layout: partition p <- pixels (2p, 2p+1);
                # per-partition runs are 256 contiguous elems (512B).
                src1 = f1_d[h0:h0 + 8, :, :].rearrange(
                    "h (p e) c -> p h (e c)", p=128)
                nc.sync.dma_start(out=stag1[:, :, :], in_=src1)
                src2 = f2_d[h0:h0 + 8, :, :].rearrange(
                    "h (p e) c -> p h (e c)", p=128)
                nc.sync.dma_start(out=stag2[:, :, :], in_=src2)

                f1tb = f1tp.tile([128, 8 * 256], F16, tag="f1t")
                f1t_blocks[blk] = f1tb
                for half in range(2):
                    pst = pstp.tile([128, 8, 128], F16, tag="pst")
                    for q in range(8):       # q = (hl, e): tile of row h0+hl
                        hl = half * 4 + q // 2
                        e = q % 2
                        nc.tensor.transpose(
                            pst[:, q, :],
                            stag1[:, hl, e * 128:(e + 1) * 128],
                            ident[:, :])
                    # de-interleave even/odd pixels: elem (hl,e,k)->hl*256+2k+e
                    dst1 = f1tb[:, half * 1024:(half + 1) * 1024].rearrange(
                        "c (a k e) -> c a e k", a=4, k=128, e=2)
                    src1 = pst[:, :, :].rearrange("c (a e) k -> c a e k", e=2)
                    if (blk + half) % 2 == 0:
                        nc.scalar.copy(dst1, src1)
                    else:
                        nc.vector.tensor_copy(dst1, src1)

                for half in range(2):
                    pst = pstp.tile([128, 8, 128], F16, tag="pst")
                    for q in range(8):
                        hl = half * 4 + q // 2
                        e = q % 2
                        nc.tensor.transpose(
                            pst[:, q, :],
                            stag2[:, hl, e * 128:(e + 1) * 128],
                            ident[:, :])
                    r0 = h0 + 4 + half * 4
                    dst2 = f2pT[:, r0:r0 + 4, 4:260].rearrange(
                        "c a (k e) -> c a e k", k=128, e=2)
                    src2 = pst[:, :, :].rearrange("c (a e) b -> c a e b", e=2)
                    if (blk + half) % 2 == 0:
                        nc.vector.tensor_copy(dst2, src2)
                    else:
                        nc.scalar.copy(dst2, src2)

            def stage_b(hblk):
                """Correlate 8 h rows, dump the raw band per w-half."""
                for wh in range(2):
                    E = ep.tile([128, ROW_E], F16, tag="ebuf")
                    for hp in range(HB // 2):
                        ps = pscp.tile([128, 2, 512], F32, tag="psc")
                        for r in range(2):
                            hl = hp * 2 + r
                            h = hblk * HB + hl
                            f1tb = f1t_blocks[h // 8]
                            base = (h % 8) * 256 + wh * 128
                            for A in range(4):
                                lhsT = f1tb[:, base + 32 * A:
                                            base + 32 * A + 32]
                                w0 = wh * 128 + 32 * A
                                rhs = f2pT[:, h:h + D, w0:w0 + JW]
                                nc.tensor.matmul(
                                    ps[32 * A:32 * A + 32, r, 0:NW],
                                    lhsT, rhs, start=True, stop=True,
                                    tile_position=(0, 32 * A))
                        dst = E[:, hp * 2 * NW:(hp + 1) * 2 * NW].rearrange(
                            "c (r n) -> c r n", r=2)
                        if hp % 2 == 0:
                            nc.scalar.copy(dst, ps[:, :, 0:NW])
                        else:
                            nc.vector.tensor_copy(dst, ps[:, :, 0:NW])
                    nc.sync.dma_start(out=band_d[wh, hblk, :, :],
                                      in_=E[:, :])

            # software-pipelined emission: B(k) needs A(k) and A(k+1).
            # B(k) is emitted BEFORE A(k+2) so the in-order PE queue runs
            # ready correlation matmuls instead of stalling on transposes
            # whose staging DMA is still in flight.
            stage_a(0)
            stage_a(1)
            for hblk in range(16):
                stage_b(hblk)
                if hblk + 2 < 16:
                    stage_a(hblk + 2)

    nc.compile()
    return nc


def kernel(feat1: np.ndarray, feat2: np.ndarray) -> np.ndarray:
    global _CACHED_NC
    feat1 = np.asarray(feat1, dtype=np.float16)
    feat2 = np.asarray(feat2, dtype=np.float16)
    B = feat1.shape[0]
    if _CACHED_NC is None:
        _CACHED_NC = _build()
    nc = _CACHED_NC
    in_maps = [{"feat1": np.ascontiguousarray(feat1[b]),
                "feat2": np.ascontiguousarray(feat2[b])} for b in range(B)]
    res = run_bass_kernel_spmd(nc, in_maps, core_ids=list(range(B)))
    band = np.stack([res.results[b]["band"] for b in range(B)], axis=0)
    # band[b, wh, hblk, p=(g,f), (hl, dy, j)]; pixel w = wh*128+32g+f uses
    # columns j = f..f+8 of its group's band at each dy.
    A = band.reshape(B, 2, H // HB, 4, 32, HB, D, JW)
    s = A.strides
    Cv = np.lib.stride_tricks.as_strided(
        A, shape=(B, 2, H // HB, 4, 32, HB, D, D),
        strides=(s[0], s[1], s[2], s[3], s[4] + s[7], s[5], s[6], s[7]))
    # Cv[b, wh, hblk, g, f, hl, dy, dx] -> out[b, h, w, dy*9+dx]
    out = Cv.transpose(0, 2, 5, 1, 3, 4, 6, 7)
    return np.ascontiguousarray(out, dtype=np.float32).reshape(B, H, W, 81)
